# revision 45
# baseline (speedup 1.0000x reference)
# Trainium2 Bass kernel for nn_Capsule (capsule routing with batch-axis softmax).
#
# Math:
#   u_hat[b,l,o] = sum_i u_vecs[b,i,l] * W[o,i]          (o = n*16+d, 160 outputs)
#   b=0; 3 routing iters:  c = softmax(b, axis=batch)    (couples ALL 64 batches)
#                          s[b,n,d] = sum_l c[b,n,l]*u_hat[b,l,(n,d)]
#                          out = s/sqrt(sum_d s^2 + 1e-10)
#                          b[b,n,l] = sum_d out[b,n,d]*u_hat[b,l,(n,d)]   (iters 0,1)
#
# Strategy: FULLY REPLICATED across the 8 cores — every core computes the whole
# problem, so there is zero cross-core communication (collectives through the
# emulated NRT layer cost seconds; replicated compute costs ~1ms).  Inputs are
# pre-cast to fp16 on the host (halves HBM traffic; measured end-to-end rel err
# ~1.3e-3 vs the fp32 reference, well inside the 2e-2 gate).
#
# Per-core schedule:
#   Pass A: stream u_vecs (fp16, 128MB), project on PE (fp16 matmuls, fp32
#           PSUM), ACT-drain to fp16 batch-pair stages (640B store chunks),
#           spill u_hat to DRAM.  Iter-0's uniform softmax (c = 1/64) is
#           folded in as PE ones-matmuls over the staged u_hat.
#   Pass B/C: stream u_hat back (42MB each); per 128-seq chunk compute
#           logits b = sum_d out*u_hat (fp16 mul + tree-adds on DVE),
#           softmax over the batch axis (all 64 batches are local!), then
#           s-accumulation on PE into PSUM banks packed 9 batches per bank
#           (3 partition bases x 3 free slots; PE psum writes must start
#           at partition 0/32/64).  The loop is software-pipelined so the
#           ACT exp of chunk k overlaps the DVE mul/tree of chunk k+1;
#           exp runs per capsule with bias = -max (a per-partition scalar
#           for fixed n) and accum_out as the softmax denominator, which
#           keeps the max-subtract and the sum off the DVE entirely.

import sys
import functools

import numpy as np

sys.path.insert(0, "/opt/trn_rl_repo")

B = 64           # global batch
KC = 4           # input-dim chunks of 128 (512 total)
L = 2048         # sequence
NCAP = 10        # capsules (n)
DCAP = 16        # capsule dim (d)
O = NCAP * DCAP  # 160
NCORES = 8
EPS = 1e-10
ALPHA0 = 1.0 / 64.0  # iter-0 uniform softmax weight


def _build_nc(b_sz=B, l_sz=L, num_devices=NCORES, phases="full"):
    import concourse.mybir as mybir
    import concourse.tile as tile
    from concourse import bacc
    from concourse.tile import add_dep_helper
    from contextlib import ExitStack

    f16 = mybir.dt.float16
    f32 = mybir.dt.float32
    Alu = mybir.AluOpType
    Act = mybir.ActivationFunctionType
    X = mybir.AxisListType.X

    ct_n = l_sz // 128          # number of 128-seq chunks
    ng = ct_n // 2              # projection groups of 2 chunks
    assert b_sz <= 64 and b_sz % 2 == 0
    n_pbank = min(7, (b_sz + 8) // 9)
    # GpSimd runs tensor ops at 0.42 efficiency with no fp16 2x (~3.2x
    # slower than DVE), its long chains stall the per-chunk pipeline, and
    # it has no PSUM port — so it does no work here.
    gp_set = set()

    nc = bacc.Bacc(trn_type="TRN2", num_devices=num_devices)

    uv_d = nc.declare_dram_parameter("uv", [b_sz, KC, 128, l_sz], f16,
                                     isOutput=False)
    wt_d = nc.declare_dram_parameter("wt", [KC, 128, O], f16, isOutput=False)
    msk_d = nc.declare_dram_parameter("msk", [NCAP, O], f32, isOutput=False)
    out_d = nc.declare_dram_parameter("out", [b_sz, NCAP, DCAP], f32,
                                      isOutput=True)

    with tile.TileContext(nc) as tc:
        ctx = ExitStack()
        consts = ctx.enter_context(tc.tile_pool(name="consts", bufs=1))
        uvp = ctx.enter_context(tc.tile_pool(name="uvp", bufs=4))
        stp = ctx.enter_context(tc.tile_pool(name="stp", bufs=12))
        rsp = ctx.enter_context(tc.tile_pool(name="rsp", bufs=2))
        wkd = ctx.enter_context(tc.tile_pool(name="wkd", bufs=1))   # DVE scratch
        b1p = ctx.enter_context(tc.tile_pool(name="b1p", bufs=2))   # logits ring
        sxp = ctx.enter_context(tc.tile_pool(name="sxp", bufs=3))   # softmax smalls
        sqp = ctx.enter_context(tc.tile_pool(name="sqp", bufs=1))   # squash smalls
        obp = ctx.enter_context(tc.tile_pool(name="obp", bufs=1))   # out bcast
        psb = ctx.enter_context(tc.tile_pool(name="psb", bufs=1, space="PSUM"))
        ps_dum = ctx.enter_context(tc.tile_pool(name="ps_dum", bufs=1,
                                                space="PSUM"))
        dramp = ctx.enter_context(tc.tile_pool(name="dramp", bufs=1,
                                               space="DRAM"))

        # ---- DRAM scratch ----
        # u_hat, chunk-major: [ct][l%128][b][o] so pass-B/C loads are one big
        # contiguous run per partition.
        uh_d = dramp.tile([ct_n, 128, b_sz, O], f16, name="uh", tag="uh")
        ob_dr = [dramp.tile([b_sz, NCAP, DCAP], f16, name=f"ob{i}",
                            tag=f"ob{i}") for i in range(2)]
        s1_dr = dramp.tile([b_sz, O], f32, name="s1d", tag="s1d")

        # ---- constants ----
        wt_sb = consts.tile([128, KC, O], f16)
        nc.sync.dma_start(out=wt_sb, in_=wt_d.rearrange("k p o -> p k o"))
        one_sb = consts.tile([1, 2], f16)
        nc.vector.memset(one_sb, 1.0)
        ones128 = consts.tile([128, 1], f16)
        nc.vector.memset(ones128, 1.0)
        # mask10[n, (n', d)] = 1 where n' == n (diagonal-block extractor)
        mask10 = consts.tile([NCAP, O], f32)
        nc.sync.dma_start(out=mask10, in_=msk_d[:, :])

        # dummy bank: row 0 = PE-observe scratch, partitions 32..41 hold the
        # s-accumulator of batch 63 (the 8th PSUM bank we don't have), and
        # partition 64 holds the iter-0 s1 accumulator (it must live OUTSIDE
        # the 7-bank projection rotation: with 8 projection groups per batch
        # an in-rotation s1 bank is re-used for projection while still open).
        dumP = ps_dum.tile([74, 512], f32, tag="dum")
        dum_ctr = [0]

        # 7 manually-managed PSUM banks: pass A rotates projection/s1 scratch
        # through them; passes B/C use them as persistent s-accumulators.
        banks = [psb.tile([128, 512], f32, tag=f"bank{t}", name=f"bank{t}")
                 for t in range(7)]
        bank_ctr = [0]

        def next_bank():
            t = bank_ctr[0] % 7
            bank_ctr[0] += 1
            return banks[t]

        def pe_observe(src_ap):
            """Dummy matmul reading src_ap so the PE picks up the producer's
            semaphore here; real matmuls then don't need that wait."""
            n = src_ap.free_size()
            j = dum_ctr[0]
            dum_ctr[0] += n
            assert dum_ctr[0] <= 512
            return nc.tensor.matmul(
                dumP[0:1, j:j + n], lhsT=one_sb[0:1, 0:1], rhs=src_ap,
                start=True, stop=True)

        const_obs = [pe_observe(wt_sb[0:1, 0, 0:1]),
                     pe_observe(one_sb[0:1, 1:2])]

        def order_after(inst, obs):
            for o in obs:
                add_dep_helper(inst.ins, o.ins, sync=False,
                               reason="after observe-dummy")

        # s-accumulator packing (iters 1, 2): bank t holds batches 9t..9t+8:
        # batch g at partition base 32*(g//3), free slot (g%3)*160.  PE psum
        # writes must start at partition 0/32/64, hence this packing.
        def s_slot(b):
            t, g = divmod(b, 9)
            if t >= n_pbank:
                return dumP[32:42, 0:O]
            base, slot = 32 * (g // 3), g % 3
            return banks[t][base:base + NCAP, slot * O:(slot + 1) * O]

        def s_flags(b):
            """(first, last) slot within this bank's partition-base region:
            only the first may open the PSUM accumulation group (start) and
            only the last closes it (stop) — start pending-zeroes the whole
            2KB bank row for the touched partitions."""
            t, g = divmod(b, 9)
            if t >= n_pbank:
                return True, True
            nb = min(9, b_sz - 9 * t)
            in_base = [x for x in range(nb) if x // 3 == g // 3]
            return g == min(in_base), g == max(in_base)

        def squash_emit(it, sd_all, nrow, cols, alpha, last):
            """squash sd_all [nrow, cols, 16] (f32) -> fp16 bounce + bcast
            (iters 0,1) or final fp32 output (iter 2).  Layout of sd_all rows
            x cols is (b, n) for iter 0 and (n, b) for iters 1/2."""
            sq = sqp.tile([nrow, cols, DCAP], f32, tag="sq")
            nc.vector.tensor_mul(sq, sd_all, sd_all)
            ssq = sqp.tile([nrow, cols], f32, tag="ssq")
            nc.vector.tensor_reduce(ssq, sq, axis=X, op=Alu.add)
            ssqe = sqp.tile([nrow, cols], f32, tag="ssqe")
            nc.vector.tensor_scalar(out=ssqe, in0=ssq,
                                    scalar1=float(alpha * alpha),
                                    scalar2=EPS, op0=Alu.mult, op1=Alu.add)
            srt = sqp.tile([nrow, cols], f32, tag="srt")
            nc.scalar.sqrt(srt, ssqe)
            rno = sqp.tile([nrow, cols], f32, tag="rno")
            nc.vector.reciprocal(rno, srt)
            rno2 = sqp.tile([nrow, cols], f32, tag="rno2")
            nc.vector.tensor_scalar(out=rno2, in0=rno, scalar1=float(alpha),
                                    scalar2=0.0, op0=Alu.mult, op1=Alu.add)
            ob = sqp.tile([nrow, cols, DCAP], f32 if last else f16,
                          tag="obq")
            nc.vector.tensor_mul(
                ob, sd_all,
                rno2.unsqueeze(2).to_broadcast((nrow, cols, DCAP)))
            if last:
                nc.sync.dma_start(
                    out=out_d.rearrange("b n d -> n b d"), in_=ob)
                return None
            if it == 0:
                st = nc.sync.dma_start(out=ob_dr[it], in_=ob)  # rows (b, n)
            else:
                st = nc.sync.dma_start(
                    out=ob_dr[it].rearrange("b n d -> n b d"), in_=ob)
            outb = obp.tile([128, b_sz * O], f16, tag="outb")
            ld = nc.sync.dma_start(
                out=outb,
                in_=ob_dr[it].rearrange("b n d -> (b n d)").unsqueeze(0)
                    .partition_broadcast(128))
            add_dep_helper(ld.ins, st.ins, sync=True,
                           reason="outb bcast after ob bounce store")
            return outb

        # =========== Pass A: projection + iter-0 s1 accumulation ===========
        uh_st = [[] for _ in range(ng)]               # uh_d store insts per g
        s1_st = []                                    # s1_dr store insts
        # stage tiles hold a PAIR of batches so each uh_d store moves 640B
        # contiguous chunks (DMA pays 2x below 512B per descriptor)
        stage_cur = [None] * ng
        for b in range(b_sz):
            uv_t = uvp.tile([128, KC, l_sz], f16, tag="uv")
            nc.sync.dma_start(out=uv_t,
                              in_=uv_d[b].rearrange("k p l -> p k l"))
            uv_obs = [pe_observe(uv_t[0:1, 0, 0:1])]
            # iter-0 shortcut s1 = sum_l u_hat accumulates at partition 64
            # of the dummy bank via ones-matmuls on the staged u_hat chunks
            # (delayed one group so the PE never waits on the ACT drain).
            s1ps = dumP[64:65, 0:O]
            pend_s1 = []

            def flush_s1(pend):
                for (st_t, jj, g_) in pend:
                    for j in range(2):
                        ct = 2 * g_ + j
                        nc.tensor.matmul(
                            s1ps, lhsT=ones128, rhs=st_t[:, j, jj, :],
                            start=(ct == 0), stop=(ct == ct_n - 1),
                            skip_group_check=True)

            # projection
            for g in range(ng):
                ps = next_bank()[:, 0:2 * O].rearrange(
                    "p (c o) -> p c o", c=2)
                for j in range(2):
                    ct = 2 * g + j
                    for kc in range(KC):
                        mm = nc.tensor.matmul(
                            ps[:, j, :],
                            lhsT=uv_t[:, kc, ct * 128:(ct + 1) * 128],
                            rhs=wt_sb[:, kc, :],
                            start=(j == 0 and kc == 0),
                            stop=(j == 1 and kc == KC - 1))
                        if kc == 0:
                            order_after(mm, uv_obs + const_obs)
                if b % 2 == 0:
                    stage_cur[g] = stp.tile([128, 2, 2, O], f16, tag="stage",
                                            name="stage")
                stage = stage_cur[g]
                # ACT drains (GpSimd has no PSUM port on real hardware)
                nc.scalar.copy(stage[:, :, b % 2, :], ps)
                if b % 2 == 1:
                    uh_st[g].append(nc.sync.dma_start(
                        out=uh_d[2 * g:2 * g + 2, :, b - 1:b + 1, :].rearrange(
                            "c p b o -> p c b o"),
                        in_=stage))
                flush_s1(pend_s1)
                pend_s1 = [(stage, b % 2, g)]
            flush_s1(pend_s1)
            s1row = rsp.tile([1, O], f32, tag="s1row")
            nc.scalar.copy(s1row, s1ps)
            s1_st.append(nc.sync.dma_start(out=s1_dr[b:b + 1, :], in_=s1row))

        s1_sb = consts.tile([b_sz, O], f32)   # s1 rows, partition = b
        s1_ld = nc.sync.dma_start(out=s1_sb, in_=s1_dr[:, :])
        for st in s1_st:
            add_dep_helper(s1_ld.ins, st.ins, sync=True,
                           reason="s1 load after row stores")
        sd0 = s1_sb.rearrange("b (n d) -> b n d", n=NCAP)
        outb = squash_emit(0, sd0, b_sz, NCAP, ALPHA0, last=False)

        # =========== routing iterations 1, 2 ===========
        n_iter = {"A": 0, "AB": 1}.get(phases, 2)
        for it in range(n_iter):
            last = (it == 1)
            outb_v = outb.rearrange("p (b o) -> p b o", b=b_sz)
            def lift(ct):
                uh_t = uvp.tile([128, b_sz, O], f16, tag="uv", name="uh_t")
                uh_ld = nc.sync.dma_start(out=uh_t, in_=uh_d[ct])
                for st in uh_st[ct // 2]:
                    add_dep_helper(uh_ld.ins, st.ins, sync=True,
                                   reason="uh load after pass-A stores")
                uh_obs = [pe_observe(uh_t[0:1, 0, 0:1])]
                prod = wkd.tile([128, b_sz, O], f16, tag="prod", name="prod")
                nc.vector.tensor_mul(prod, uh_t, outb_v)
                pv = prod.rearrange("p b (n d) -> p b n d", n=NCAP)
                t8 = wkd.tile([128, b_sz, NCAP, 8], f16, tag="t8", name="t8")
                nc.vector.tensor_add(t8, pv[:, :, :, 0:8], pv[:, :, :, 8:16])
                t4 = wkd.tile([128, b_sz, NCAP, 4], f16, tag="t4", name="t4")
                nc.vector.tensor_add(t4, t8[:, :, :, 0:4], t8[:, :, :, 4:8])
                t2 = wkd.tile([128, b_sz, NCAP, 2], f16, tag="t2", name="t2")
                nc.vector.tensor_add(t2, t4[:, :, :, 0:2], t4[:, :, :, 2:4])
                b1 = b1p.tile([128, b_sz, NCAP], f16, tag="b1", name="b1")
                nc.vector.tensor_add(b1, t2[:, :, :, 0], t2[:, :, :, 1])
                return (ct, uh_t, uh_obs, b1)

            def smax_head(state):
                ct, uh_t, uh_obs, b1 = state
                m_t = sxp.tile([128, NCAP], f16, tag="m", name="m")
                nc.vector.tensor_reduce(
                    m_t, b1.rearrange("p b n -> p n b"), axis=X, op=Alu.max)
                negm = sxp.tile([128, NCAP], f32, tag="negm", name="negm")
                nc.vector.tensor_scalar(out=negm, in0=m_t, scalar1=-1.0,
                                        scalar2=0.0, op0=Alu.mult, op1=Alu.add)
                p_t = sxp.tile([128, b_sz, NCAP], f16, tag="pt", name="p")
                S_t = sxp.tile([128, NCAP], f32, tag="S", name="S")
                # per-capsule exp: bias is a per-partition scalar for fixed n,
                # and accum_out yields the softmax denominator for free
                for n in range(NCAP):
                    nc.scalar.activation(p_t[:, :, n], b1[:, :, n], Act.Exp,
                                         bias=negm[:, n:n + 1],
                                         accum_out=S_t[:, n:n + 1])
                return p_t, S_t

            def smax_tail(state, ps):
                ct, uh_t, uh_obs, b1 = state
                p_t, S_t = ps
                rS = sxp.tile([128, NCAP], f32, tag="rS", name="rS")
                nc.vector.reciprocal(rS, S_t)
                rS16 = sxp.tile([128, NCAP], f16, tag="rS16", name="rS16")
                nc.vector.tensor_copy(rS16, rS)
                c_t = sxp.tile([128, b_sz, NCAP], f16, tag="c", name="c")
                nc.vector.tensor_mul(
                    c_t, p_t,
                    rS16.unsqueeze(1).to_broadcast((128, b_sz, NCAP)))
                for b in range(b_sz):
                    first, lasts = s_flags(b)
                    # the sim's global psum group-check mis-derives the
                    # partition for base!=0 psum APs (assumes 16KB partition
                    # stride); the per-memref pending-zero data model is
                    # correct, so skip the check for those slots only.
                    skip = (b // 9 >= n_pbank) or (b % 9) // 3 > 0
                    mm = nc.tensor.matmul(
                        s_slot(b), lhsT=c_t[:, b, :], rhs=uh_t[:, b, :],
                        start=(ct == 0 and first),
                        stop=(ct == ct_n - 1 and lasts),
                        skip_group_check=skip)
                    if b == 0:
                        order_after(mm, uh_obs)

            prev = None
            for ct in range(ct_n + 1):
                ps_prev = smax_head(prev) if prev is not None else None
                cur = lift(ct) if ct < ct_n else None
                if prev is not None:
                    smax_tail(prev, ps_prev)
                prev = cur
            # drain s accumulators: masked diagonal extraction, one op per
            # (bank, partition-base) covering up to 3 packed batches
            sd_all = sqp.tile([NCAP, b_sz, DCAP], f32, tag="sda")
            b = 0
            while b < b_sz:
                t, g = divmod(b, 9)
                if t >= n_pbank:
                    k, src_ap = 1, dumP[32:42, 0:O].rearrange(
                        "p (k o) -> p k o", k=1)
                else:
                    base = 32 * (g // 3)
                    k = min(3, min(9, b_sz - 9 * t) - g)
                    src_ap = banks[t][base:base + NCAP, 0:k * O].rearrange(
                        "p (k o) -> p k o", k=k)
                mmb = sqp.tile([NCAP, 3, O], f32, tag="mmb")
                nc.vector.tensor_mul(
                    mmb[:, 0:k, :], src_ap,
                    mask10.unsqueeze(1).to_broadcast((NCAP, k, O)))
                nc.vector.tensor_reduce(
                    sd_all[:, b:b + k, :],
                    mmb[:, 0:k, :].rearrange("p k (n d) -> p k d n", n=NCAP),
                    axis=X, op=Alu.add)
                b += k
            outb = squash_emit(1 + it, sd_all, NCAP, b_sz, 1.0, last=last)

        ctx.close()
    nc.finalize()
    return nc


@functools.lru_cache(maxsize=2)
def _get_nc(b_sz=B, l_sz=L, num_devices=NCORES):
    return _build_nc(b_sz, l_sz, num_devices)


def _prep_inputs(u_vecs, W, b_sz=B, l_sz=L):
    uv16 = np.ascontiguousarray(u_vecs, dtype=np.float32) \
        .reshape(b_sz, KC, 128, l_sz).astype(np.float16)
    wt16 = np.ascontiguousarray(
        W[:, :, 0].astype(np.float32).T.reshape(KC, 128, O)).astype(np.float16)
    return uv16, wt16


def _mask10():
    msk = np.zeros((NCAP, O), np.float32)
    for n in range(NCAP):
        msk[n, n * DCAP:(n + 1) * DCAP] = 1.0
    return msk


def kernel(u_vecs: np.ndarray, W: np.ndarray) -> np.ndarray:
    from concourse.bass_utils import run_bass_kernel_spmd

    uv16, wt16 = _prep_inputs(u_vecs, W)
    nc = _get_nc()
    in_maps = [{"uv": uv16, "wt": wt16, "msk": _mask10()}
               for _ in range(NCORES)]
    res = run_bass_kernel_spmd(nc, in_maps, core_ids=list(range(NCORES)))
    return res.results[0]["out"].astype(np.float32)
